# revision 25
# baseline (speedup 1.0000x reference)
"""Multi-head attention forward on 8 TRN2 NeuronCores — v4.

Problem: x[2,2048,1024] @ {Wq,Wk,Wv}[1024,1024] (+bias) -> 16 heads of 64,
softmax(QK^T/8)V per head, concat -> @Wo[1024,1024] + bo.

Sharding: tensor-parallel over d_hid. Core c owns 2 heads (128 dims).
Host sums the 8 partial out projections and adds bo.

v4 changes vs v2 (241026ns measured):
  - scores: zero-padded-Q full-K=128 matmuls (qtz_h = q chunk with the
    other head's rows zeroed; pads memset once on DVE at prologue).
    v2's K=64 head pairs relied on LDWEIGHTS pull-ahead row-tile
    concurrency which only fired for 43/128 pairs (two ~97ns loads
    rarely fit in the previous mm's 216ns stream); full-K mms pipeline
    deterministically at ~216ns each.
  - fp8 DoubleRow was tried for the projections (1.81x PE measured) but
    W-quantization is a coherent model perturbation that doesn't
    average out: all-fp8 rel err 2.5e-2 > the 2e-2 gate. Projections
    stay f16 (KQK8 env flag re-enables fp8 Q/K for experiments).
  - batch-transition stall fixed: v2 lost ~6us at the b0->b1 seam
    because b1's projection drains were scheduled behind b0-qc3's
    serial stage->recip->broadcast->muls chain in the DVE queue, and
    the gpsimd queue head-of-line blocked on the broadcast round-trip.
    fill1 moved ~20 slots earlier and the last qcs of each batch use
    compressed stage/norm/outp offsets.
  - input DMAs issued across gpsimd+sync+scalar in first-chunk-first
    order (v2 serialized 16 x^T loads on sync at ~0.9us per issue
    before the first matmul could start).
  - tail out-DMAs spread across sync/gpsimd/scalar.
"""

import os
import numpy as np

B, S, D = 2, 2048, 1024
NCORES = 8
HSLICE = D // NCORES          # 128 = 2 heads x 64
KT_PROJ = 8                   # d_in contraction tiles for projections
QH = 512                      # q chunk
NQC = S // QH                 # 4 q chunks per batch
NKT = S // 128                # 16 k tiles per batch
NPAIR = NKT // 2              # 8 ki pairs (et tiles hold 2 ki)

_cache = {}


def _build():
    import concourse.bacc as bacc
    import concourse.tile as tile
    from concourse import mybir

    f32 = mybir.dt.float32
    f16 = mybir.dt.float16
    AF = mybir.ActivationFunctionType

    nc = bacc.Bacc("TRN2", target_bir_lowering=False, debug=False,
                   num_devices=NCORES)

    xt_d = nc.dram_tensor("xt", [D, B * S], f16, kind="ExternalInput").ap()
    # wq/wk/wv pre-rearranged on host to [128, 8*128] (k-tiles side by
    # side) so each loads with ONE contiguous descriptor.
    wq_d = nc.dram_tensor("wq", [128, D], f16, kind="ExternalInput").ap()
    wk_d = nc.dram_tensor("wk", [128, D], f16, kind="ExternalInput").ap()
    wv_d = nc.dram_tensor("wv", [128, D], f16, kind="ExternalInput").ap()
    bq_d = nc.dram_tensor("bq", [HSLICE, 1], f32, kind="ExternalInput").ap()
    bk_d = nc.dram_tensor("bk", [HSLICE, 1], f32, kind="ExternalInput").ap()
    bv_d = nc.dram_tensor("bv", [HSLICE, 1], f32, kind="ExternalInput").ap()
    wo_d = nc.dram_tensor("wo", [HSLICE, D], f16, kind="ExternalInput").ap()
    idt_d = nc.dram_tensor("idt", [128, 128], f16, kind="ExternalInput").ap()
    out_d = nc.dram_tensor("out", [B * S, D], f16, kind="ExternalOutput").ap()

    with tile.TileContext(nc) as tc:
        with (
            tc.tile_pool(name="wpool", bufs=1) as wpool,
            tc.tile_pool(name="xtp", bufs=1) as xtp,
            tc.tile_pool(name="qk", bufs=1) as qkp,
            tc.tile_pool(name="vap", bufs=1) as vap,
            tc.tile_pool(name="etp", bufs=6) as etp,
            tc.tile_pool(name="ctxp", bufs=1) as ctxp,
            tc.tile_pool(name="stp", bufs=2) as stp,
            tc.tile_pool(name="normp", bufs=2) as normp,
            tc.tile_pool(name="ostp", bufs=4) as ostp,
            tc.tile_pool(name="psS", bufs=2, space="PSUM") as psS,
            tc.tile_pool(name="psC", bufs=1, space="PSUM") as psC,
            tc.tile_pool(name="psP", bufs=2, space="PSUM") as psP,
        ):
            # ---- weights + x^T, spread across gpsimd/sync/scalar issue
            # queues, first-needed first (the first proj chunk needs all
            # 8 k-tiles of tokens 0:512 plus wq). ----
            def wtiles(tag, src, eng, bias_d, bias_tag):
                t = wpool.tile([128, KT_PROJ * HSLICE], f16, tag=tag,
                               name=tag)
                eng.dma_start(t[:], src[:])
                b_t = wpool.tile([128, 1], f32, tag=bias_tag, name=bias_tag)
                nc.gpsimd.dma_start(b_t[:], bias_d[:])
                lst = [t[:, ki * HSLICE:(ki + 1) * HSLICE]
                       for ki in range(KT_PROJ)]
                return lst, b_t

            wq_t, bq_t = wtiles("wq", wq_d, nc.gpsimd, bq_d, "bq")
            wk_t, bk_t = wtiles("wk", wk_d, nc.sync, bk_d, "bk")
            wv_t, bv_t = wtiles("wv", wv_d, nc.scalar, bv_d, "bv")

            xts = []
            for ki in range(KT_PROJ):
                t = xtp.tile([128, B * S], f16, tag=f"xt{ki}", name=f"xt{ki}")
                xts.append(t)
            ring3 = [nc.gpsimd, nc.sync, nc.scalar]
            # first q chunk of batch 0 (the critical path)
            for ki in range(KT_PROJ):
                ring3[ki % 3].dma_start(
                    xts[ki][:, 0:QH], xt_d[ki * 128:(ki + 1) * 128, 0:QH])
            # rest of batch 0
            for ki in range(KT_PROJ):
                ring3[ki % 3].dma_start(
                    xts[ki][:, QH:S], xt_d[ki * 128:(ki + 1) * 128, QH:S])
            idt = wpool.tile([128, 128], f16, tag="idt")
            nc.sync.dma_start(idt[:], idt_d[:])
            wo_t = wpool.tile([128, D], f16, tag="wo")
            nc.sync.dma_start(wo_t[:], wo_d[:])
            # batch 1 (needed from ~slot 20)
            for ki in range(KT_PROJ):
                ring3[ki % 3].dma_start(
                    xts[ki][:, S:2 * S], xt_d[ki * 128:(ki + 1) * 128, S:2 * S])

            # ---- per-(batch, chunk) projection tiles, explicit tags.
            # qtz: zero-padded per head; pads memset once on DVE. ----
            qtz = {}   # (b, qc, h) -> [128, 512] f16
            kt = {}    # (b, c) -> [128, 512] f16
            vt = {}    # (b, c) -> [128, 512] f16
            for b in range(B):
                for c in range(NQC):
                    for h in range(2):
                        qtz[(b, c, h)] = qkp.tile(
                            [128, QH], f16, tag=f"qtz{b}{c}{h}",
                            name=f"qtz{b}{c}{h}")
                    kt[(b, c)] = qkp.tile([128, QH], f16, tag=f"kt{b}{c}",
                                          name=f"kt{b}{c}")
                    vt[(b, c)] = qkp.tile([128, QH], f16, tag=f"vt{b}{c}",
                                          name=f"vt{b}{c}")
            for (b, c, h) in sorted(qtz):
                t = qtz[(b, c, h)]
                pad = t[64:128, :] if h == 0 else t[0:64, :]
                nc.vector.memset(pad, 0.0)

            va = [{} for _ in range(B)]     # ki -> [128, 130] f16
            et = [{} for _ in range(B)]     # (qc, p) -> [128, 2048] f16
            ctx_ps = [{} for _ in range(B)]  # (qc, h) -> [65, 512] f32 psum
            stg = [{} for _ in range(B)]    # (qc, h) -> [65, 512] f32
            ctxT = {}                       # (b, qc) -> [128, 512] f16
            bc_t = {}                       # (b, qc, h) -> [64, 512] f32

            def proj_step(b, which, c):
                """One projection chunk: 8 matmuls + DVE drain w/ bias."""
                ps = psP.tile([128, QH], f32, tag="pp", name="pp")
                w_t = {"q": wq_t, "k": wk_t, "v": wv_t}[which]
                col0 = b * S + c * QH
                for ki in range(KT_PROJ):
                    nc.tensor.matmul(ps[:], w_t[ki][:],
                                     xts[ki][:, col0:col0 + QH],
                                     start=(ki == 0), stop=(ki == KT_PROJ - 1))
                if which == "q":
                    for h in range(2):
                        nc.vector.tensor_scalar_add(
                            qtz[(b, c, h)][h * 64:(h + 1) * 64, :],
                            ps[h * 64:(h + 1) * 64, :],
                            bq_t[h * 64:(h + 1) * 64, 0:1])
                else:
                    dst = kt[(b, c)] if which == "k" else vt[(b, c)]
                    b_t = bk_t if which == "k" else bv_t
                    nc.vector.tensor_scalar_add(dst[:], ps[:], b_t[:, 0:1])

            def vaug_step(b, p):
                """Transpose V tiles ki=2p,2p+1 into f16 [128, 130] va
                tiles: per head h a [65]-col block = 64 V^T dims + a ones
                column (psum row 64 of ctx = softmax denominator)."""
                for j in range(2):
                    ki = 2 * p + j
                    c = ki // 4
                    vat = vap.tile([128, 130], f16, tag=f"va{b}{ki}",
                                   name=f"va{b}{ki}")
                    va[b][ki] = vat
                    ones_v = vat[:].rearrange("p (h m) -> p h m", h=2)
                    nc.gpsimd.memset(ones_v[:, :, 64:65], 1.0)
                    tp = psP.tile([128, 128], f16, tag="pp", name="tp")
                    nc.tensor.transpose(
                        tp[:], vt[(b, c)][:, (ki % 4) * 128:(ki % 4 + 1) * 128],
                        idt[:])
                    src = tp[:].rearrange("p (h m) -> p h m", h=2)
                    dstv = vat[:].rearrange(
                        "p (h m) -> p h m", h=2)[:, :, 0:64]
                    nc.vector.tensor_copy(dstv, src)

            def score_step(b, qc, ki):
                sc = psS.tile([128, 1024], f32, tag="sc", name="sc")
                c, kk = ki // 4, (ki % 4) * 128
                for h in range(2):
                    nc.tensor.matmul(
                        sc[:, h * 512:(h + 1) * 512],
                        kt[(b, c)][:, kk:kk + 128],
                        qtz[(b, qc, h)][:],
                        start=True, stop=True)
                p, j = ki // 2, ki % 2
                if j == 0:
                    et[b][(qc, p)] = etp.tile([128, 2048], f16, tag="et",
                                              name="et")
                nc.scalar.activation(
                    et[b][(qc, p)][:, j * 1024:(j + 1) * 1024], sc[:], AF.Exp)

            def ctx_step(b, qc, p):
                ett = et[b][(qc, p)]
                if p == 0:
                    for h in range(2):
                        ctx_ps[b][(qc, h)] = psC.tile([65, 512], f32,
                                                      tag=f"c{h}", name=f"c{h}")
                for j in range(2):
                    ki = 2 * p + j
                    for h in range(2):
                        nc.tensor.matmul(
                            ctx_ps[b][(qc, h)][:],
                            va[b][ki][:, h * 65:(h + 1) * 65],
                            ett[:, j * 1024 + h * 512:j * 1024 + (h + 1) * 512],
                            start=(ki == 0), stop=(ki == NKT - 1))

            def stage_step(b, qc, tail=False):
                """Drain ctx psum (frees psC for qc+1). In the tail the
                h1 copy rides the idle Act engine so the two heads drain
                in parallel."""
                for h in range(2):
                    st = stp.tile([65, 512], f32, tag=f"st{h}", name=f"st{h}")
                    stg[b][(qc, h)] = st
                    if tail and h == 1:
                        nc.scalar.activation(
                            st[:], ctx_ps[b][(qc, h)][0:65, :], AF.Copy)
                    else:
                        nc.vector.tensor_copy(st[:], ctx_ps[b][(qc, h)][0:65, :])

            def normA_step(b, qc):
                """Reciprocal + broadcast of the denominators; the DMA hop
                moves the den row to partition 0 (DVE can't cross
                partitions) and the broadcast round-trip stays off the
                DVE queue."""
                for h in range(2):
                    st = stg[b][(qc, h)]
                    r0 = normp.tile([1, QH], f32, tag=f"r0{h}", name=f"r0{h}")
                    nc.gpsimd.dma_start(r0[:], st[64:65, :])
                    rc = normp.tile([1, QH], f32, tag=f"rc{h}", name=f"rc{h}")
                    nc.vector.reciprocal_approx_fast(rc[:], r0[:])
                    bc = normp.tile([64, QH], f32, tag=f"bc{h}", name=f"bc{h}")
                    nc.gpsimd.partition_broadcast(bc[:], rc[:])
                    bc_t[(b, qc, h)] = bc

            def normB_step(b, qc):
                t = ctxp.tile([128, QH], f16, tag=f"ctxT{b}{qc}",
                              name=f"ctxT{b}{qc}")
                ctxT[(b, qc)] = t
                for h in range(2):
                    nc.vector.tensor_mul(
                        out=t[h * 64:(h + 1) * 64, :],
                        in0=stg[b][(qc, h)][0:64, :],
                        in1=bc_t[(b, qc, h)][:])

            def outp_step(b, qc, st_i, tail=False):
                row0 = b * S + qc * QH + st_i * 128
                for half in range(2):
                    if tail and half == 1:
                        po = psS.tile([128, 512], f32, tag="sc", name="po")
                    else:
                        po = psP.tile([128, 512], f32, tag="pp", name="po")
                    nc.tensor.matmul(
                        po[:],
                        ctxT[(b, qc)][:, st_i * 128:(st_i + 1) * 128],
                        wo_t[:, half * 512:(half + 1) * 512],
                        start=True, stop=True)
                    ot = ostp.tile([128, 512], f16, tag="ost", name="ost")
                    if tail and half == 1:
                        # Act is idle after the last exp — split the psum
                        # drains across Act/DVE so they don't serialize.
                        nc.scalar.activation(ot[:], po[:], AF.Copy)
                    else:
                        nc.vector.tensor_copy(ot[:], po[:])
                    if tail:
                        eng = [nc.sync, nc.gpsimd, nc.scalar][
                            (2 * st_i + half) % 3]
                    else:
                        eng = nc.sync if half == 0 else nc.gpsimd
                    eng.dma_start(
                        out_d[row0:row0 + 128, half * 512:(half + 1) * 512],
                        ot[:])

            # ---- global slot schedule ----
            from collections import defaultdict
            actions = defaultdict(list)   # g -> [(prio, fn)]

            for b in range(B):
                base = b * 64
                for qc in range(NQC):
                    tail = (b == B - 1 and qc == NQC - 1)
                    # last qc of each batch (and b1-qc2) has no following
                    # scores to hide its stage->norm->outp chain behind:
                    # compress the offsets so the PE isn't starved at the
                    # batch seams.
                    endq = tail or (qc == NQC - 1) or (b == B - 1 and
                                                       qc == NQC - 2)
                    for ki in range(NKT):
                        g = base + qc * 16 + ki
                        actions[g].append(
                            (0, (lambda b=b, qc=qc, ki=ki:
                                 score_step(b, qc, ki))))
                    for p in range(NPAIR):
                        off = 6 + 2 * p if not (tail and p >= 6) else 11 + p
                        actions[base + qc * 16 + off].append(
                            (3, (lambda b=b, qc=qc, p=p: ctx_step(b, qc, p))))
                    # stage must not precede ctx p7 (offset 20 non-tail,
                    # 18 tail) — it reads the ctx psum mid-accumulation
                    # otherwise.
                    o_st, o_nA, o_nB, o_out, ostride = (
                        (20, 22, 24, 27, 2) if not endq else
                        (18, 19, 20, 22, 1) if tail else
                        (20, 21, 22, 23, 1))
                    actions[base + qc * 16 + o_st].append(
                        (3.5, (lambda b=b, qc=qc, t=tail: stage_step(b, qc, t))))
                    actions[base + qc * 16 + o_nA].append(
                        (2, (lambda b=b, qc=qc: normA_step(b, qc))))
                    actions[base + qc * 16 + o_nB].append(
                        (2, (lambda b=b, qc=qc: normB_step(b, qc))))
                    for st_i in range(4):
                        g = base + qc * 16 + o_out + ostride * st_i
                        actions[g].append(
                            (4, (lambda b=b, qc=qc, s=st_i, t=tail:
                                 outp_step(b, qc, s, t))))

            def F(step, *a):
                return lambda: step(*a)

            fill0 = {0: F(proj_step, 0, "k", 1), 1: F(proj_step, 0, "k", 2),
                     2: F(vaug_step, 0, 1), 3: F(proj_step, 0, "v", 1),
                     4: F(vaug_step, 0, 2), 5: F(proj_step, 0, "k", 3),
                     6: F(vaug_step, 0, 3), 7: F(proj_step, 0, "v", 2),
                     8: F(vaug_step, 0, 4), 9: F(proj_step, 0, "q", 1),
                     10: F(vaug_step, 0, 5), 11: F(proj_step, 0, "v", 3),
                     12: F(vaug_step, 0, 6), 13: F(vaug_step, 0, 7),
                     14: F(proj_step, 0, "q", 2), 15: F(proj_step, 0, "q", 3)}
            fill1 = {20: F(proj_step, 1, "q", 0), 21: F(proj_step, 1, "k", 0),
                     23: F(proj_step, 1, "v", 0), 24: F(vaug_step, 1, 0),
                     25: F(proj_step, 1, "k", 1), 26: F(proj_step, 1, "k", 2),
                     28: F(vaug_step, 1, 1), 29: F(proj_step, 1, "v", 1),
                     30: F(vaug_step, 1, 2), 31: F(proj_step, 1, "k", 3),
                     32: F(vaug_step, 1, 3), 33: F(proj_step, 1, "v", 2),
                     34: F(vaug_step, 1, 4), 35: F(proj_step, 1, "q", 1),
                     36: F(vaug_step, 1, 5), 37: F(proj_step, 1, "v", 3),
                     38: F(vaug_step, 1, 6), 39: F(vaug_step, 1, 7),
                     40: F(proj_step, 1, "q", 2), 41: F(proj_step, 1, "q", 3)}
            for g, fn in list(fill0.items()) + list(fill1.items()):
                actions[g].append((5, fn))

            # ---- emit: prologue then slots in order ----
            proj_step(0, "q", 0)
            proj_step(0, "k", 0)
            proj_step(0, "v", 0)
            vaug_step(0, 0)
            for g in range(max(actions) + 1):
                for _, fn in sorted(actions[g], key=lambda x: x[0]):
                    fn()

    nc.compile()
    return nc


def _get_nc():
    if "nc" not in _cache:
        _cache["nc"] = _build()
    return _cache["nc"]


def kernel(x, Wq, bq, Wk, bk, Wv, bv, Wo, bo):
    from concourse.bass_utils import run_bass_kernel_spmd

    nc = _get_nc()

    x = np.ascontiguousarray(np.asarray(x, dtype=np.float32))
    xt = np.ascontiguousarray(x.reshape(B * S, D).T)          # [D, B*S]
    idt = np.eye(128, dtype=np.float16)

    def wprep(W, sl, scale=1.0):
        """[1024, 128] slice -> [128, 8*128] with k-tiles side by side:
        out[p, ki*128 + m] = W[ki*128 + p, sl][m] (one contiguous DMA)."""
        w = np.asarray(W, np.float32)[:, sl] * scale
        return np.ascontiguousarray(
            w.reshape(KT_PROJ, 128, HSLICE).transpose(1, 0, 2)
            .reshape(128, KT_PROJ * HSLICE)).astype(np.float16)

    in_maps = []
    for c in range(NCORES):
        sl = slice(c * HSLICE, (c + 1) * HSLICE)
        in_maps.append({
            "xt": xt.astype(np.float16),
            "wq": wprep(Wq, sl, 1.0 / 8.0),
            "wk": wprep(Wk, sl),
            "wv": wprep(Wv, sl),
            "bq": (np.asarray(bq, np.float32)[sl] / 8.0).reshape(HSLICE, 1),
            "bk": np.asarray(bk, np.float32)[sl].reshape(HSLICE, 1),
            "bv": np.asarray(bv, np.float32)[sl].reshape(HSLICE, 1),
            "wo": np.ascontiguousarray(
                np.asarray(Wo, np.float32)[sl, :]).astype(np.float16),
            "idt": idt,
        })

    res = run_bass_kernel_spmd(nc, in_maps, core_ids=list(range(NCORES)),
                               trace=bool(int(os.environ.get("KTRACE", "0"))))
    _cache["last_result"] = res
    acc = res.results[0]["out"].astype(np.float32)
    for c in range(1, NCORES):
        acc += res.results[c]["out"].astype(np.float32)
    acc += np.asarray(bo, np.float32)[None, :]
    return acc.reshape(B, S, D)


# revision 26
# speedup vs baseline: 1.0179x; 1.0179x over previous
"""Multi-head attention forward on 8 TRN2 NeuronCores — v4.

Problem: x[2,2048,1024] @ {Wq,Wk,Wv}[1024,1024] (+bias) -> 16 heads of 64,
softmax(QK^T/8)V per head, concat -> @Wo[1024,1024] + bo.

Sharding: tensor-parallel over d_hid. Core c owns 2 heads (128 dims).
Host sums the 8 partial out projections and adds bo.

v4 changes vs v2 (241026ns measured):
  - scores: zero-padded-Q full-K=128 matmuls (qtz_h = q chunk with the
    other head's rows zeroed; pads memset once on DVE at prologue).
    v2's K=64 head pairs relied on LDWEIGHTS pull-ahead row-tile
    concurrency which only fired for 43/128 pairs (two ~97ns loads
    rarely fit in the previous mm's 216ns stream); full-K mms pipeline
    deterministically at ~216ns each.
  - fp8 DoubleRow was tried for the projections (1.81x PE measured) but
    W-quantization is a coherent model perturbation that doesn't
    average out: all-fp8 rel err 2.5e-2 > the 2e-2 gate. Projections
    stay f16 (KQK8 env flag re-enables fp8 Q/K for experiments).
  - batch-transition stall fixed: v2 lost ~6us at the b0->b1 seam
    because b1's projection drains were scheduled behind b0-qc3's
    serial stage->recip->broadcast->muls chain in the DVE queue, and
    the gpsimd queue head-of-line blocked on the broadcast round-trip.
    fill1 moved ~20 slots earlier and the last qcs of each batch use
    compressed stage/norm/outp offsets.
  - input DMAs issued across gpsimd+sync+scalar in first-chunk-first
    order (v2 serialized 16 x^T loads on sync at ~0.9us per issue
    before the first matmul could start).
  - tail out-DMAs spread across sync/gpsimd/scalar.
"""

import os
import numpy as np

B, S, D = 2, 2048, 1024
NCORES = 8
HSLICE = D // NCORES          # 128 = 2 heads x 64
KT_PROJ = 8                   # d_in contraction tiles for projections
QH = 512                      # q chunk
NQC = S // QH                 # 4 q chunks per batch
NKT = S // 128                # 16 k tiles per batch
NPAIR = NKT // 2              # 8 ki pairs (et tiles hold 2 ki)

_cache = {}


def _build():
    import concourse.bacc as bacc
    import concourse.tile as tile
    from concourse import mybir

    f32 = mybir.dt.float32
    f16 = mybir.dt.float16
    AF = mybir.ActivationFunctionType

    nc = bacc.Bacc("TRN2", target_bir_lowering=False, debug=False,
                   num_devices=NCORES)

    xt_d = nc.dram_tensor("xt", [D, B * S], f16, kind="ExternalInput").ap()
    # wq/wk/wv pre-rearranged on host to [128, 8*128] (k-tiles side by
    # side) so each loads with ONE contiguous descriptor.
    wq_d = nc.dram_tensor("wq", [128, D], f16, kind="ExternalInput").ap()
    wk_d = nc.dram_tensor("wk", [128, D], f16, kind="ExternalInput").ap()
    wv_d = nc.dram_tensor("wv", [128, D], f16, kind="ExternalInput").ap()
    bq_d = nc.dram_tensor("bq", [HSLICE, 1], f32, kind="ExternalInput").ap()
    bk_d = nc.dram_tensor("bk", [HSLICE, 1], f32, kind="ExternalInput").ap()
    bv_d = nc.dram_tensor("bv", [HSLICE, 1], f32, kind="ExternalInput").ap()
    wo_d = nc.dram_tensor("wo", [HSLICE, D], f16, kind="ExternalInput").ap()
    idt_d = nc.dram_tensor("idt", [128, 128], f16, kind="ExternalInput").ap()
    out_d = nc.dram_tensor("out", [B * S, D], f16, kind="ExternalOutput").ap()

    with tile.TileContext(nc) as tc:
        with (
            tc.tile_pool(name="wpool", bufs=1) as wpool,
            tc.tile_pool(name="xtp", bufs=1) as xtp,
            tc.tile_pool(name="qk", bufs=1) as qkp,
            tc.tile_pool(name="vap", bufs=1) as vap,
            tc.tile_pool(name="etp", bufs=6) as etp,
            tc.tile_pool(name="ctxp", bufs=1) as ctxp,
            tc.tile_pool(name="stp", bufs=2) as stp,
            tc.tile_pool(name="normp", bufs=2) as normp,
            tc.tile_pool(name="ostp", bufs=4) as ostp,
            tc.tile_pool(name="psS", bufs=2, space="PSUM") as psS,
            tc.tile_pool(name="psC", bufs=1, space="PSUM") as psC,
            tc.tile_pool(name="psP", bufs=2, space="PSUM") as psP,
        ):
            # ---- weights + x^T, spread across gpsimd/sync/scalar issue
            # queues, first-needed first (the first proj chunk needs all
            # 8 k-tiles of tokens 0:512 plus wq). ----
            def wtiles(tag, src, eng, bias_d, bias_tag):
                t = wpool.tile([128, KT_PROJ * HSLICE], f16, tag=tag,
                               name=tag)
                eng.dma_start(t[:], src[:])
                b_t = wpool.tile([128, 1], f32, tag=bias_tag, name=bias_tag)
                nc.gpsimd.dma_start(b_t[:], bias_d[:])
                lst = [t[:, ki * HSLICE:(ki + 1) * HSLICE]
                       for ki in range(KT_PROJ)]
                return lst, b_t

            wq_t, bq_t = wtiles("wq", wq_d, nc.gpsimd, bq_d, "bq")
            wk_t, bk_t = wtiles("wk", wk_d, nc.sync, bk_d, "bk")
            wv_t, bv_t = wtiles("wv", wv_d, nc.scalar, bv_d, "bv")

            xts = []
            for ki in range(KT_PROJ):
                t = xtp.tile([128, B * S], f16, tag=f"xt{ki}", name=f"xt{ki}")
                xts.append(t)
            ring3 = [nc.gpsimd, nc.sync, nc.scalar]
            # first q chunk of batch 0 (the critical path)
            for ki in range(KT_PROJ):
                ring3[ki % 3].dma_start(
                    xts[ki][:, 0:QH], xt_d[ki * 128:(ki + 1) * 128, 0:QH])
            # rest of batch 0
            for ki in range(KT_PROJ):
                ring3[ki % 3].dma_start(
                    xts[ki][:, QH:S], xt_d[ki * 128:(ki + 1) * 128, QH:S])
            idt = wpool.tile([128, 128], f16, tag="idt")
            nc.sync.dma_start(idt[:], idt_d[:])
            wo_t = wpool.tile([128, D], f16, tag="wo")
            nc.sync.dma_start(wo_t[:], wo_d[:])
            # batch 1 (needed from ~slot 20)
            for ki in range(KT_PROJ):
                ring3[ki % 3].dma_start(
                    xts[ki][:, S:2 * S], xt_d[ki * 128:(ki + 1) * 128, S:2 * S])

            # ---- per-(batch, chunk) projection tiles, explicit tags.
            # qtz: zero-padded per head; pads memset once on DVE. ----
            qtz = {}   # (b, qc, h) -> [128, 512] f16
            kt = {}    # (b, c) -> [128, 512] f16
            vt = {}    # (b, c) -> [128, 512] f16
            for b in range(B):
                for c in range(NQC):
                    for h in range(2):
                        qtz[(b, c, h)] = qkp.tile(
                            [128, QH], f16, tag=f"qtz{b}{c}{h}",
                            name=f"qtz{b}{c}{h}")
                    kt[(b, c)] = qkp.tile([128, QH], f16, tag=f"kt{b}{c}",
                                          name=f"kt{b}{c}")
                    vt[(b, c)] = qkp.tile([128, QH], f16, tag=f"vt{b}{c}",
                                          name=f"vt{b}{c}")
            for (b, c, h) in sorted(qtz):
                t = qtz[(b, c, h)]
                pad = t[64:128, :] if h == 0 else t[0:64, :]
                nc.vector.memset(pad, 0.0)

            va = [{} for _ in range(B)]     # ki -> [128, 130] f16
            et = [{} for _ in range(B)]     # (qc, p) -> [128, 2048] f16
            ctx_ps = [{} for _ in range(B)]  # (qc, h) -> [65, 512] f32 psum
            stg = [{} for _ in range(B)]    # (qc, h) -> [65, 512] f32
            ctxT = {}                       # (b, qc) -> [128, 512] f16
            bc_t = {}                       # (b, qc, h) -> [64, 512] f32

            def proj_step(b, which, c):
                """One projection chunk: 8 matmuls + DVE drain w/ bias."""
                ps = psP.tile([128, QH], f32, tag="pp", name="pp")
                w_t = {"q": wq_t, "k": wk_t, "v": wv_t}[which]
                col0 = b * S + c * QH
                for ki in range(KT_PROJ):
                    nc.tensor.matmul(ps[:], w_t[ki][:],
                                     xts[ki][:, col0:col0 + QH],
                                     start=(ki == 0), stop=(ki == KT_PROJ - 1))
                if which == "q":
                    for h in range(2):
                        nc.vector.tensor_scalar_add(
                            qtz[(b, c, h)][h * 64:(h + 1) * 64, :],
                            ps[h * 64:(h + 1) * 64, :],
                            bq_t[h * 64:(h + 1) * 64, 0:1])
                else:
                    dst = kt[(b, c)] if which == "k" else vt[(b, c)]
                    b_t = bk_t if which == "k" else bv_t
                    nc.vector.tensor_scalar_add(dst[:], ps[:], b_t[:, 0:1])

            def vaug_step(b, p):
                """Transpose V tiles ki=2p,2p+1 into f16 [128, 130] va
                tiles: per head h a [65]-col block = 64 V^T dims + a ones
                column (psum row 64 of ctx = softmax denominator)."""
                for j in range(2):
                    ki = 2 * p + j
                    c = ki // 4
                    vat = vap.tile([128, 130], f16, tag=f"va{b}{ki}",
                                   name=f"va{b}{ki}")
                    va[b][ki] = vat
                    ones_v = vat[:].rearrange("p (h m) -> p h m", h=2)
                    nc.gpsimd.memset(ones_v[:, :, 64:65], 1.0)
                    tp = psP.tile([128, 128], f16, tag="pp", name="tp")
                    nc.tensor.transpose(
                        tp[:], vt[(b, c)][:, (ki % 4) * 128:(ki % 4 + 1) * 128],
                        idt[:])
                    src = tp[:].rearrange("p (h m) -> p h m", h=2)
                    dstv = vat[:].rearrange(
                        "p (h m) -> p h m", h=2)[:, :, 0:64]
                    nc.vector.tensor_copy(dstv, src)

            def score_step(b, qc, ki):
                sc = psS.tile([128, 1024], f32, tag="sc", name="sc")
                c, kk = ki // 4, (ki % 4) * 128
                for h in range(2):
                    nc.tensor.matmul(
                        sc[:, h * 512:(h + 1) * 512],
                        kt[(b, c)][:, kk:kk + 128],
                        qtz[(b, qc, h)][:],
                        start=True, stop=True)
                p, j = ki // 2, ki % 2
                if j == 0:
                    et[b][(qc, p)] = etp.tile([128, 2048], f16, tag="et",
                                              name="et")
                nc.scalar.activation(
                    et[b][(qc, p)][:, j * 1024:(j + 1) * 1024], sc[:], AF.Exp)

            def ctx_step(b, qc, p):
                ett = et[b][(qc, p)]
                if p == 0:
                    for h in range(2):
                        ctx_ps[b][(qc, h)] = psC.tile([65, 512], f32,
                                                      tag=f"c{h}", name=f"c{h}")
                for j in range(2):
                    ki = 2 * p + j
                    for h in range(2):
                        nc.tensor.matmul(
                            ctx_ps[b][(qc, h)][:],
                            va[b][ki][:, h * 65:(h + 1) * 65],
                            ett[:, j * 1024 + h * 512:j * 1024 + (h + 1) * 512],
                            start=(ki == 0), stop=(ki == NKT - 1))

            def stage_step(b, qc):
                """Drain ctx psum (frees psC for qc+1)."""
                for h in range(2):
                    st = stp.tile([65, 512], f32, tag=f"st{h}", name=f"st{h}")
                    stg[b][(qc, h)] = st
                    nc.vector.tensor_copy(st[:], ctx_ps[b][(qc, h)][0:65, :])

            def normA_step(b, qc):
                """Reciprocal + broadcast of the denominators; the DMA hop
                moves the den row to partition 0 (DVE can't cross
                partitions) and the broadcast round-trip stays off the
                DVE queue."""
                for h in range(2):
                    st = stg[b][(qc, h)]
                    r0 = normp.tile([1, QH], f32, tag=f"r0{h}", name=f"r0{h}")
                    nc.gpsimd.dma_start(r0[:], st[64:65, :])
                    rc = normp.tile([1, QH], f32, tag=f"rc{h}", name=f"rc{h}")
                    nc.vector.reciprocal_approx_fast(rc[:], r0[:])
                    bc = normp.tile([64, QH], f32, tag=f"bc{h}", name=f"bc{h}")
                    nc.gpsimd.partition_broadcast(bc[:], rc[:])
                    bc_t[(b, qc, h)] = bc

            def normB_step(b, qc):
                t = ctxp.tile([128, QH], f16, tag=f"ctxT{b}{qc}",
                              name=f"ctxT{b}{qc}")
                ctxT[(b, qc)] = t
                for h in range(2):
                    nc.vector.tensor_mul(
                        out=t[h * 64:(h + 1) * 64, :],
                        in0=stg[b][(qc, h)][0:64, :],
                        in1=bc_t[(b, qc, h)][:])

            def outp_step(b, qc, st_i, tail=False):
                row0 = b * S + qc * QH + st_i * 128
                for half in range(2):
                    if tail and half == 1:
                        po = psS.tile([128, 512], f32, tag="sc", name="po")
                    else:
                        po = psP.tile([128, 512], f32, tag="pp", name="po")
                    nc.tensor.matmul(
                        po[:],
                        ctxT[(b, qc)][:, st_i * 128:(st_i + 1) * 128],
                        wo_t[:, half * 512:(half + 1) * 512],
                        start=True, stop=True)
                    ot = ostp.tile([128, 512], f16, tag="ost", name="ost")
                    nc.vector.tensor_copy(ot[:], po[:])
                    if tail:
                        eng = [nc.sync, nc.gpsimd, nc.scalar][
                            (2 * st_i + half) % 3]
                    else:
                        eng = nc.sync if half == 0 else nc.gpsimd
                    eng.dma_start(
                        out_d[row0:row0 + 128, half * 512:(half + 1) * 512],
                        ot[:])

            # ---- global slot schedule ----
            from collections import defaultdict
            actions = defaultdict(list)   # g -> [(prio, fn)]

            for b in range(B):
                base = b * 64
                for qc in range(NQC):
                    tail = (b == B - 1 and qc == NQC - 1)
                    # last qc of each batch (and b1-qc2) has no following
                    # scores to hide its stage->norm->outp chain behind:
                    # compress the offsets so the PE isn't starved at the
                    # batch seams.
                    endq = tail or (qc == NQC - 1) or (b == B - 1 and
                                                       qc == NQC - 2)
                    for ki in range(NKT):
                        g = base + qc * 16 + ki
                        actions[g].append(
                            (0, (lambda b=b, qc=qc, ki=ki:
                                 score_step(b, qc, ki))))
                    for p in range(NPAIR):
                        off = 6 + 2 * p if not (tail and p >= 6) else 11 + p
                        actions[base + qc * 16 + off].append(
                            (3, (lambda b=b, qc=qc, p=p: ctx_step(b, qc, p))))
                    # stage must not precede ctx p7 (offset 20 non-tail,
                    # 18 tail) — it reads the ctx psum mid-accumulation
                    # otherwise.
                    o_st, o_nA, o_nB, o_out, ostride = (
                        (20, 22, 24, 27, 2) if not endq else
                        (18, 19, 20, 22, 1) if tail else
                        (20, 21, 22, 23, 1))
                    actions[base + qc * 16 + o_st].append(
                        (3.5, (lambda b=b, qc=qc: stage_step(b, qc))))
                    actions[base + qc * 16 + o_nA].append(
                        (2, (lambda b=b, qc=qc: normA_step(b, qc))))
                    actions[base + qc * 16 + o_nB].append(
                        (2, (lambda b=b, qc=qc: normB_step(b, qc))))
                    for st_i in range(4):
                        g = base + qc * 16 + o_out + ostride * st_i
                        actions[g].append(
                            (4, (lambda b=b, qc=qc, s=st_i, t=tail:
                                 outp_step(b, qc, s, t))))

            def F(step, *a):
                return lambda: step(*a)

            fill0 = {0: F(proj_step, 0, "k", 1), 1: F(proj_step, 0, "k", 2),
                     2: F(vaug_step, 0, 1), 3: F(proj_step, 0, "v", 1),
                     4: F(vaug_step, 0, 2), 5: F(proj_step, 0, "k", 3),
                     6: F(vaug_step, 0, 3), 7: F(proj_step, 0, "v", 2),
                     8: F(vaug_step, 0, 4), 9: F(proj_step, 0, "q", 1),
                     10: F(vaug_step, 0, 5), 11: F(proj_step, 0, "v", 3),
                     12: F(vaug_step, 0, 6), 13: F(vaug_step, 0, 7),
                     14: F(proj_step, 0, "q", 2), 15: F(proj_step, 0, "q", 3)}
            fill1 = {20: F(proj_step, 1, "q", 0), 21: F(proj_step, 1, "k", 0),
                     23: F(proj_step, 1, "v", 0), 24: F(vaug_step, 1, 0),
                     25: F(proj_step, 1, "k", 1), 26: F(proj_step, 1, "k", 2),
                     28: F(vaug_step, 1, 1), 29: F(proj_step, 1, "v", 1),
                     30: F(vaug_step, 1, 2), 31: F(proj_step, 1, "k", 3),
                     32: F(vaug_step, 1, 3), 33: F(proj_step, 1, "v", 2),
                     34: F(vaug_step, 1, 4), 35: F(proj_step, 1, "q", 1),
                     36: F(vaug_step, 1, 5), 37: F(proj_step, 1, "v", 3),
                     38: F(vaug_step, 1, 6), 39: F(vaug_step, 1, 7),
                     40: F(proj_step, 1, "q", 2), 41: F(proj_step, 1, "q", 3)}
            for g, fn in list(fill0.items()) + list(fill1.items()):
                actions[g].append((5, fn))

            # ---- emit: prologue then slots in order ----
            proj_step(0, "q", 0)
            proj_step(0, "k", 0)
            proj_step(0, "v", 0)
            vaug_step(0, 0)
            for g in range(max(actions) + 1):
                for _, fn in sorted(actions[g], key=lambda x: x[0]):
                    fn()

    nc.compile()
    return nc


def _get_nc():
    if "nc" not in _cache:
        _cache["nc"] = _build()
    return _cache["nc"]


def kernel(x, Wq, bq, Wk, bk, Wv, bv, Wo, bo):
    from concourse.bass_utils import run_bass_kernel_spmd

    nc = _get_nc()

    x = np.ascontiguousarray(np.asarray(x, dtype=np.float32))
    xt = np.ascontiguousarray(x.reshape(B * S, D).T)          # [D, B*S]
    idt = np.eye(128, dtype=np.float16)

    def wprep(W, sl, scale=1.0):
        """[1024, 128] slice -> [128, 8*128] with k-tiles side by side:
        out[p, ki*128 + m] = W[ki*128 + p, sl][m] (one contiguous DMA)."""
        w = np.asarray(W, np.float32)[:, sl] * scale
        return np.ascontiguousarray(
            w.reshape(KT_PROJ, 128, HSLICE).transpose(1, 0, 2)
            .reshape(128, KT_PROJ * HSLICE)).astype(np.float16)

    in_maps = []
    for c in range(NCORES):
        sl = slice(c * HSLICE, (c + 1) * HSLICE)
        in_maps.append({
            "xt": xt.astype(np.float16),
            "wq": wprep(Wq, sl, 1.0 / 8.0),
            "wk": wprep(Wk, sl),
            "wv": wprep(Wv, sl),
            "bq": (np.asarray(bq, np.float32)[sl] / 8.0).reshape(HSLICE, 1),
            "bk": np.asarray(bk, np.float32)[sl].reshape(HSLICE, 1),
            "bv": np.asarray(bv, np.float32)[sl].reshape(HSLICE, 1),
            "wo": np.ascontiguousarray(
                np.asarray(Wo, np.float32)[sl, :]).astype(np.float16),
            "idt": idt,
        })

    res = run_bass_kernel_spmd(nc, in_maps, core_ids=list(range(NCORES)),
                               trace=bool(int(os.environ.get("KTRACE", "0"))))
    _cache["last_result"] = res
    acc = res.results[0]["out"].astype(np.float32)
    for c in range(1, NCORES):
        acc += res.results[c]["out"].astype(np.float32)
    acc += np.asarray(bo, np.float32)[None, :]
    return acc.reshape(B, S, D)


# revision 27
# speedup vs baseline: 1.0248x; 1.0068x over previous
"""Multi-head attention forward on 8 TRN2 NeuronCores — v4.

Problem: x[2,2048,1024] @ {Wq,Wk,Wv}[1024,1024] (+bias) -> 16 heads of 64,
softmax(QK^T/8)V per head, concat -> @Wo[1024,1024] + bo.

Sharding: tensor-parallel over d_hid. Core c owns 2 heads (128 dims).
Host sums the 8 partial out projections and adds bo.

v4 changes vs v2 (241026ns measured):
  - scores: zero-padded-Q full-K=128 matmuls (qtz_h = q chunk with the
    other head's rows zeroed; pads memset once on DVE at prologue).
    v2's K=64 head pairs relied on LDWEIGHTS pull-ahead row-tile
    concurrency which only fired for 43/128 pairs (two ~97ns loads
    rarely fit in the previous mm's 216ns stream); full-K mms pipeline
    deterministically at ~216ns each.
  - fp8 DoubleRow was tried for the projections (1.81x PE measured) but
    W-quantization is a coherent model perturbation that doesn't
    average out: all-fp8 rel err 2.5e-2 > the 2e-2 gate. Projections
    stay f16 (KQK8 env flag re-enables fp8 Q/K for experiments).
  - batch-transition stall fixed: v2 lost ~6us at the b0->b1 seam
    because b1's projection drains were scheduled behind b0-qc3's
    serial stage->recip->broadcast->muls chain in the DVE queue, and
    the gpsimd queue head-of-line blocked on the broadcast round-trip.
    fill1 moved ~20 slots earlier and the last qcs of each batch use
    compressed stage/norm/outp offsets.
  - input DMAs issued across gpsimd+sync+scalar in first-chunk-first
    order (v2 serialized 16 x^T loads on sync at ~0.9us per issue
    before the first matmul could start).
  - tail out-DMAs spread across sync/gpsimd/scalar.
"""

import os
import numpy as np

B, S, D = 2, 2048, 1024
NCORES = 8
HSLICE = D // NCORES          # 128 = 2 heads x 64
KT_PROJ = 8                   # d_in contraction tiles for projections
QH = 512                      # q chunk
NQC = S // QH                 # 4 q chunks per batch
NKT = S // 128                # 16 k tiles per batch
NPAIR = NKT // 2              # 8 ki pairs (et tiles hold 2 ki)

_cache = {}


def _build():
    import concourse.bacc as bacc
    import concourse.tile as tile
    from concourse import mybir

    f32 = mybir.dt.float32
    f16 = mybir.dt.float16
    AF = mybir.ActivationFunctionType

    nc = bacc.Bacc("TRN2", target_bir_lowering=False, debug=False,
                   num_devices=NCORES)

    xt_d = nc.dram_tensor("xt", [D, B * S], f16, kind="ExternalInput").ap()
    # wq/wk/wv pre-rearranged on host to [128, 8*128] (k-tiles side by
    # side) so each loads with ONE contiguous descriptor.
    wq_d = nc.dram_tensor("wq", [128, D], f16, kind="ExternalInput").ap()
    wk_d = nc.dram_tensor("wk", [128, D], f16, kind="ExternalInput").ap()
    wv_d = nc.dram_tensor("wv", [128, D], f16, kind="ExternalInput").ap()
    bq_d = nc.dram_tensor("bq", [HSLICE, 1], f32, kind="ExternalInput").ap()
    bk_d = nc.dram_tensor("bk", [HSLICE, 1], f32, kind="ExternalInput").ap()
    bv_d = nc.dram_tensor("bv", [HSLICE, 1], f32, kind="ExternalInput").ap()
    wo_d = nc.dram_tensor("wo", [HSLICE, D], f16, kind="ExternalInput").ap()
    idt_d = nc.dram_tensor("idt", [128, 128], f16, kind="ExternalInput").ap()
    out_d = nc.dram_tensor("out", [B * S, D], f16, kind="ExternalOutput").ap()

    with tile.TileContext(nc) as tc:
        with (
            tc.tile_pool(name="wpool", bufs=1) as wpool,
            tc.tile_pool(name="xtp", bufs=1) as xtp,
            tc.tile_pool(name="qk", bufs=1) as qkp,
            tc.tile_pool(name="vap", bufs=1) as vap,
            tc.tile_pool(name="etp", bufs=6) as etp,
            tc.tile_pool(name="ctxp", bufs=1) as ctxp,
            tc.tile_pool(name="stp", bufs=2) as stp,
            tc.tile_pool(name="normp", bufs=2) as normp,
            tc.tile_pool(name="ostp", bufs=4) as ostp,
            tc.tile_pool(name="psS", bufs=2, space="PSUM") as psS,
            tc.tile_pool(name="psC", bufs=1, space="PSUM") as psC,
            tc.tile_pool(name="psP", bufs=2, space="PSUM") as psP,
        ):
            # ---- weights + x^T, spread across gpsimd/sync/scalar issue
            # queues, first-needed first (the first proj chunk needs all
            # 8 k-tiles of tokens 0:512 plus wq). ----
            def wtiles(tag, src, eng, bias_d, bias_tag):
                t = wpool.tile([128, KT_PROJ * HSLICE], f16, tag=tag,
                               name=tag)
                eng.dma_start(t[:], src[:])
                b_t = wpool.tile([128, 1], f32, tag=bias_tag, name=bias_tag)
                nc.gpsimd.dma_start(b_t[:], bias_d[:])
                lst = [t[:, ki * HSLICE:(ki + 1) * HSLICE]
                       for ki in range(KT_PROJ)]
                return lst, b_t

            wq_t, bq_t = wtiles("wq", wq_d, nc.gpsimd, bq_d, "bq")
            wk_t, bk_t = wtiles("wk", wk_d, nc.sync, bk_d, "bk")
            wv_t, bv_t = wtiles("wv", wv_d, nc.scalar, bv_d, "bv")

            xts = []
            for ki in range(KT_PROJ):
                t = xtp.tile([128, B * S], f16, tag=f"xt{ki}", name=f"xt{ki}")
                xts.append(t)
            ring3 = [nc.gpsimd, nc.sync, nc.scalar]
            # first q chunk of batch 0 (the critical path)
            for ki in range(KT_PROJ):
                ring3[ki % 3].dma_start(
                    xts[ki][:, 0:QH], xt_d[ki * 128:(ki + 1) * 128, 0:QH])
            # rest of batch 0
            for ki in range(KT_PROJ):
                ring3[ki % 3].dma_start(
                    xts[ki][:, QH:S], xt_d[ki * 128:(ki + 1) * 128, QH:S])
            idt = wpool.tile([128, 128], f16, tag="idt")
            nc.sync.dma_start(idt[:], idt_d[:])
            wo_t = wpool.tile([128, D], f16, tag="wo")
            nc.sync.dma_start(wo_t[:], wo_d[:])
            # batch 1 (needed from ~slot 20)
            for ki in range(KT_PROJ):
                ring3[ki % 3].dma_start(
                    xts[ki][:, S:2 * S], xt_d[ki * 128:(ki + 1) * 128, S:2 * S])

            # ---- per-(batch, chunk) projection tiles, explicit tags.
            # qtz: zero-padded per head; pads memset once on DVE. ----
            qtz = {}   # (b, qc, h) -> [128, 512] f16
            kt = {}    # (b, c) -> [128, 512] f16
            vt = {}    # (b, c) -> [128, 512] f16
            for b in range(B):
                for c in range(NQC):
                    for h in range(2):
                        qtz[(b, c, h)] = qkp.tile(
                            [128, QH], f16, tag=f"qtz{b}{c}{h}",
                            name=f"qtz{b}{c}{h}")
                    kt[(b, c)] = qkp.tile([128, QH], f16, tag=f"kt{b}{c}",
                                          name=f"kt{b}{c}")
                    vt[(b, c)] = qkp.tile([128, QH], f16, tag=f"vt{b}{c}",
                                          name=f"vt{b}{c}")
            for (b, c, h) in sorted(qtz):
                t = qtz[(b, c, h)]
                pad = t[64:128, :] if h == 0 else t[0:64, :]
                nc.vector.memset(pad, 0.0)

            va = [{} for _ in range(B)]     # ki -> [128, 130] f16
            et = [{} for _ in range(B)]     # (qc, p) -> [128, 2048] f16
            ctx_ps = [{} for _ in range(B)]  # (qc, h) -> [65, 512] f32 psum
            stg = [{} for _ in range(B)]    # (qc, h) -> [65, 512] f32
            ctxT = {}                       # (b, qc) -> [128, 512] f16
            bc_t = {}                       # (b, qc, h) -> [64, 512] f32

            def proj_step(b, which, c):
                """One projection chunk: 8 matmuls + DVE drain w/ bias."""
                ps = psP.tile([128, QH], f32, tag="pp", name="pp")
                w_t = {"q": wq_t, "k": wk_t, "v": wv_t}[which]
                col0 = b * S + c * QH
                for ki in range(KT_PROJ):
                    nc.tensor.matmul(ps[:], w_t[ki][:],
                                     xts[ki][:, col0:col0 + QH],
                                     start=(ki == 0), stop=(ki == KT_PROJ - 1))
                if which == "q":
                    for h in range(2):
                        nc.vector.tensor_scalar_add(
                            qtz[(b, c, h)][h * 64:(h + 1) * 64, :],
                            ps[h * 64:(h + 1) * 64, :],
                            bq_t[h * 64:(h + 1) * 64, 0:1])
                else:
                    dst = kt[(b, c)] if which == "k" else vt[(b, c)]
                    b_t = bk_t if which == "k" else bv_t
                    nc.vector.tensor_scalar_add(dst[:], ps[:], b_t[:, 0:1])

            def vaug_step(b, p):
                """Transpose V tiles ki=2p,2p+1 into f16 [128, 130] va
                tiles: per head h a [65]-col block = 64 V^T dims + a ones
                column (psum row 64 of ctx = softmax denominator)."""
                for j in range(2):
                    ki = 2 * p + j
                    c = ki // 4
                    vat = vap.tile([128, 130], f16, tag=f"va{b}{ki}",
                                   name=f"va{b}{ki}")
                    va[b][ki] = vat
                    ones_v = vat[:].rearrange("p (h m) -> p h m", h=2)
                    nc.gpsimd.memset(ones_v[:, :, 64:65], 1.0)
                    tp = psP.tile([128, 128], f16, tag="pp", name="tp")
                    nc.tensor.transpose(
                        tp[:], vt[(b, c)][:, (ki % 4) * 128:(ki % 4 + 1) * 128],
                        idt[:])
                    src = tp[:].rearrange("p (h m) -> p h m", h=2)
                    dstv = vat[:].rearrange(
                        "p (h m) -> p h m", h=2)[:, :, 0:64]
                    nc.vector.tensor_copy(dstv, src)

            def score_step(b, qc, ki):
                sc = psS.tile([128, 1024], f32, tag="sc", name="sc")
                c, kk = ki // 4, (ki % 4) * 128
                for h in range(2):
                    nc.tensor.matmul(
                        sc[:, h * 512:(h + 1) * 512],
                        kt[(b, c)][:, kk:kk + 128],
                        qtz[(b, qc, h)][:],
                        start=True, stop=True)
                p, j = ki // 2, ki % 2
                if j == 0:
                    et[b][(qc, p)] = etp.tile([128, 2048], f16, tag="et",
                                              name="et")
                nc.scalar.activation(
                    et[b][(qc, p)][:, j * 1024:(j + 1) * 1024], sc[:], AF.Exp)

            def ctx_step(b, qc, p):
                ett = et[b][(qc, p)]
                if p == 0:
                    for h in range(2):
                        ctx_ps[b][(qc, h)] = psC.tile([65, 512], f32,
                                                      tag=f"c{h}", name=f"c{h}")
                for j in range(2):
                    ki = 2 * p + j
                    for h in range(2):
                        nc.tensor.matmul(
                            ctx_ps[b][(qc, h)][:],
                            va[b][ki][:, h * 65:(h + 1) * 65],
                            ett[:, j * 1024 + h * 512:j * 1024 + (h + 1) * 512],
                            start=(ki == 0), stop=(ki == NKT - 1))

            def stage_step(b, qc):
                """Drain ctx psum (frees psC for qc+1)."""
                for h in range(2):
                    st = stp.tile([65, 512], f32, tag=f"st{h}", name=f"st{h}")
                    stg[b][(qc, h)] = st
                    nc.vector.tensor_copy(st[:], ctx_ps[b][(qc, h)][0:65, :])

            def normA_step(b, qc):
                """Reciprocal + broadcast of the denominators; the DMA hop
                moves the den row to partition 0 (DVE can't cross
                partitions) and the broadcast round-trip stays off the
                DVE queue."""
                for h in range(2):
                    st = stg[b][(qc, h)]
                    r0 = normp.tile([1, QH], f32, tag=f"r0{h}", name=f"r0{h}")
                    nc.gpsimd.dma_start(r0[:], st[64:65, :])
                    rc = normp.tile([1, QH], f32, tag=f"rc{h}", name=f"rc{h}")
                    nc.vector.reciprocal_approx_fast(rc[:], r0[:])
                    bc = normp.tile([64, QH], f32, tag=f"bc{h}", name=f"bc{h}")
                    nc.gpsimd.partition_broadcast(bc[:], rc[:])
                    bc_t[(b, qc, h)] = bc

            def normB_step(b, qc):
                t = ctxp.tile([128, QH], f16, tag=f"ctxT{b}{qc}",
                              name=f"ctxT{b}{qc}")
                ctxT[(b, qc)] = t
                for h in range(2):
                    nc.vector.tensor_mul(
                        out=t[h * 64:(h + 1) * 64, :],
                        in0=stg[b][(qc, h)][0:64, :],
                        in1=bc_t[(b, qc, h)][:])

            def outp_step(b, qc, st_i, tail=False):
                row0 = b * S + qc * QH + st_i * 128
                for half in range(2):
                    if tail and half == 1:
                        po = psS.tile([128, 512], f32, tag="sc", name="po")
                    else:
                        po = psP.tile([128, 512], f32, tag="pp", name="po")
                    nc.tensor.matmul(
                        po[:],
                        ctxT[(b, qc)][:, st_i * 128:(st_i + 1) * 128],
                        wo_t[:, half * 512:(half + 1) * 512],
                        start=True, stop=True)
                    ot = ostp.tile([128, 512], f16, tag="ost", name="ost")
                    nc.vector.tensor_copy(ot[:], po[:])
                    if tail:
                        eng = [nc.sync, nc.gpsimd, nc.scalar][
                            (2 * st_i + half) % 3]
                    else:
                        eng = nc.sync if half == 0 else nc.gpsimd
                    eng.dma_start(
                        out_d[row0:row0 + 128, half * 512:(half + 1) * 512],
                        ot[:])

            # ---- global slot schedule ----
            from collections import defaultdict
            actions = defaultdict(list)   # g -> [(prio, fn)]

            for b in range(B):
                base = b * 64
                for qc in range(NQC):
                    tail = (b == B - 1 and qc == NQC - 1)
                    # last qc of each batch (and b1-qc2) has no following
                    # scores to hide its stage->norm->outp chain behind:
                    # compress the offsets so the PE isn't starved at the
                    # batch seams.
                    endq = tail or (qc == NQC - 1) or (b == B - 1 and
                                                       qc == NQC - 2)
                    for ki in range(NKT):
                        g = base + qc * 16 + ki
                        actions[g].append(
                            (0, (lambda b=b, qc=qc, ki=ki:
                                 score_step(b, qc, ki))))
                    for p in range(NPAIR):
                        off = 6 + 2 * p if not (tail and p >= 6) else 11 + p
                        actions[base + qc * 16 + off].append(
                            (3, (lambda b=b, qc=qc, p=p: ctx_step(b, qc, p))))
                    # stage must not precede ctx p7 (offset 20 non-tail,
                    # 18 tail) — it reads the ctx psum mid-accumulation
                    # otherwise.
                    o_st, o_nA, o_nB, o_out, ostride = (
                        (20, 22, 24, 27, 2) if not endq else
                        (18, 19, 20, 22, 1) if tail else
                        (20, 21, 22, 23, 1))
                    actions[base + qc * 16 + o_st].append(
                        (3.5, (lambda b=b, qc=qc: stage_step(b, qc))))
                    actions[base + qc * 16 + o_nA].append(
                        (2, (lambda b=b, qc=qc: normA_step(b, qc))))
                    actions[base + qc * 16 + o_nB].append(
                        (2, (lambda b=b, qc=qc: normB_step(b, qc))))
                    for st_i in range(4):
                        g = base + qc * 16 + o_out + ostride * st_i
                        actions[g].append(
                            (4, (lambda b=b, qc=qc, s=st_i, t=tail:
                                 outp_step(b, qc, s, t))))

            def F(step, *a):
                return lambda: step(*a)

            fill0 = {0: F(proj_step, 0, "k", 1), 1: F(proj_step, 0, "k", 2),
                     2: F(vaug_step, 0, 1), 3: F(proj_step, 0, "v", 1),
                     4: F(vaug_step, 0, 2), 5: F(proj_step, 0, "k", 3),
                     6: F(vaug_step, 0, 3), 7: F(proj_step, 0, "v", 2),
                     8: F(vaug_step, 0, 4), 9: F(proj_step, 0, "q", 1),
                     10: F(vaug_step, 0, 5), 11: F(proj_step, 0, "v", 3),
                     12: F(vaug_step, 0, 6), 13: F(vaug_step, 0, 7),
                     14: F(proj_step, 0, "q", 2), 15: F(proj_step, 0, "q", 3)}
            def HP(step, *a):
                # promote to scheduler priority ~0: the b1-qc0 projection
                # drains must beat b0-qc3's stage->recip->bcast->muls
                # chain in the DVE stream, or the first b1 scores stall
                # ~4.7us at the batch seam (the list scheduler otherwise
                # orders them after the norm chain).
                def fn():
                    with tc.high_priority():
                        step(*a)
                return fn

            fill1 = {20: HP(proj_step, 1, "q", 0), 21: HP(proj_step, 1, "k", 0),
                     23: HP(proj_step, 1, "v", 0), 24: HP(vaug_step, 1, 0),
                     25: F(proj_step, 1, "k", 1), 26: F(proj_step, 1, "k", 2),
                     28: F(vaug_step, 1, 1), 29: F(proj_step, 1, "v", 1),
                     30: F(vaug_step, 1, 2), 31: F(proj_step, 1, "k", 3),
                     32: F(vaug_step, 1, 3), 33: F(proj_step, 1, "v", 2),
                     34: F(vaug_step, 1, 4), 35: F(proj_step, 1, "q", 1),
                     36: F(vaug_step, 1, 5), 37: F(proj_step, 1, "v", 3),
                     38: F(vaug_step, 1, 6), 39: F(vaug_step, 1, 7),
                     40: F(proj_step, 1, "q", 2), 41: F(proj_step, 1, "q", 3)}
            for g, fn in list(fill0.items()) + list(fill1.items()):
                actions[g].append((5, fn))

            # ---- emit: prologue then slots in order ----
            proj_step(0, "q", 0)
            proj_step(0, "k", 0)
            proj_step(0, "v", 0)
            vaug_step(0, 0)
            for g in range(max(actions) + 1):
                for _, fn in sorted(actions[g], key=lambda x: x[0]):
                    fn()

    nc.compile()
    return nc


def _get_nc():
    if "nc" not in _cache:
        _cache["nc"] = _build()
    return _cache["nc"]


def kernel(x, Wq, bq, Wk, bk, Wv, bv, Wo, bo):
    from concourse.bass_utils import run_bass_kernel_spmd

    nc = _get_nc()

    x = np.ascontiguousarray(np.asarray(x, dtype=np.float32))
    xt = np.ascontiguousarray(x.reshape(B * S, D).T)          # [D, B*S]
    idt = np.eye(128, dtype=np.float16)

    def wprep(W, sl, scale=1.0):
        """[1024, 128] slice -> [128, 8*128] with k-tiles side by side:
        out[p, ki*128 + m] = W[ki*128 + p, sl][m] (one contiguous DMA)."""
        w = np.asarray(W, np.float32)[:, sl] * scale
        return np.ascontiguousarray(
            w.reshape(KT_PROJ, 128, HSLICE).transpose(1, 0, 2)
            .reshape(128, KT_PROJ * HSLICE)).astype(np.float16)

    in_maps = []
    for c in range(NCORES):
        sl = slice(c * HSLICE, (c + 1) * HSLICE)
        in_maps.append({
            "xt": xt.astype(np.float16),
            "wq": wprep(Wq, sl, 1.0 / 8.0),
            "wk": wprep(Wk, sl),
            "wv": wprep(Wv, sl),
            "bq": (np.asarray(bq, np.float32)[sl] / 8.0).reshape(HSLICE, 1),
            "bk": np.asarray(bk, np.float32)[sl].reshape(HSLICE, 1),
            "bv": np.asarray(bv, np.float32)[sl].reshape(HSLICE, 1),
            "wo": np.ascontiguousarray(
                np.asarray(Wo, np.float32)[sl, :]).astype(np.float16),
            "idt": idt,
        })

    res = run_bass_kernel_spmd(nc, in_maps, core_ids=list(range(NCORES)),
                               trace=bool(int(os.environ.get("KTRACE", "0"))))
    _cache["last_result"] = res
    acc = res.results[0]["out"].astype(np.float32)
    for c in range(1, NCORES):
        acc += res.results[c]["out"].astype(np.float32)
    acc += np.asarray(bo, np.float32)[None, :]
    return acc.reshape(B, S, D)
